# revision 2
# baseline (speedup 1.0000x reference)
"""DeepSeek-V3 MoE routing kernel for Trainium2 (Bass/Tile), 8-core SPMD.

v2 redesign from measured per-op engine rates:
  ACT  0.83 ns/elem (+285 fixed)  - sigmoid, Sign-select
  Pool 2.13 ns/elem (+150)        - add/sub/mult ONLY (no max/compare!)
  DVE  1.07 ns/elem (+140)        - everything incl. max8/find_index8/reduce
  Pool TENSOR_SCALAR = 14 ns/elem - NEVER use (the old kernel's bottleneck)

Group top-2 via the exact identity t2(S) = max(pair-sum levels):
pair-sums on Pool (add), pair-maxes on DVE (native max), final grouped
max-reduce on DVE.  Group/top-8 masking additive (-1e30).  Selection by
ACT Sign with nudged threshold, final top-8 + indices via DVE
max8/match_value/find_index8 on true scores.

Software-pipelined in 3 phases at iteration offsets (k, k-2, k-4) so every
cross-engine dependency has >= 1 full iteration of slack (in-order engine
queues otherwise head-of-line block on the sign->smult->max8 ladder).
"""

import numpy as np

T_FULL = 131072
E = 256
G = 8
EG = 32
N_CORES = 8
T_CORE = T_FULL // N_CORES
P = 128
BIG = 1.0e30
TB = 8  # token-chunks (of P) per tile group
W = TB * E
JG = TB * G  # group-scores per partition row
DMA_SPLIT = 2  # input DMA issued in this many chunks
SEL_STT = False  # selection via fused DVE scalar_tensor_tensor (vs ACT Sign)
TREE_DVE = True  # pair-sums (levels 2+) on DVE instead of Pool
POOL_P1 = False  # level-1 pair-sum on Pool from scores + pairsum(bias) const
GS_MR = False  # group scores via reduce + match_replace + reduce (no ladder)


def build_bass(n_tokens: int):
    from contextlib import ExitStack

    import concourse.bacc as bacc
    import concourse.mybir as mybir
    import concourse.tile as tile

    f32 = mybir.dt.float32
    u32 = mybir.dt.uint32
    A = mybir.AluOpType
    AX = mybir.AxisListType
    AF = mybir.ActivationFunctionType

    assert n_tokens % (P * TB) == 0
    n_groups = n_tokens // (P * TB)

    nc = bacc.Bacc("TRN2", target_bir_lowering=False, debug=False)

    logits_d = nc.dram_tensor("logits", [n_tokens, E], f32, kind="ExternalInput").ap()
    biasb_d = nc.dram_tensor("biasb", [P, W], f32, kind="ExternalInput").ap()
    idx_d = nc.dram_tensor("idx", [n_tokens, 8], mybir.dt.int32, kind="ExternalOutput").ap()
    vals_d = nc.dram_tensor("vals", [n_tokens, 8], f32, kind="ExternalOutput").ap()

    with tile.TileContext(nc) as tc, ExitStack() as ctx:
        setup = ctx.enter_context(tc.tile_pool(name="setup", bufs=1))
        pool_sc = ctx.enter_context(tc.tile_pool(name="sc", bufs=4))
        pool_swb = ctx.enter_context(tc.tile_pool(name="swb", bufs=2))
        pool_swbm = ctx.enter_context(tc.tile_pool(name="swbm", bufs=4))
        pool_s = ctx.enter_context(tc.tile_pool(name="s", bufs=4))
        med = ctx.enter_context(tc.tile_pool(name="med", bufs=2))
        sel = ctx.enter_context(tc.tile_pool(name="sel", bufs=6))
        out = ctx.enter_context(tc.tile_pool(name="out", bufs=6))

        biasT = setup.tile([P, W], f32)
        nc.sync.dma_start(biasT[:], biasb_d)
        pbias = setup.tile([P, JG * 16], f32)
        if POOL_P1:
            bg = biasT[:].rearrange("p (g e) -> p g e", g=JG)
            nc.gpsimd.tensor_tensor(
                pbias[:].rearrange("p (g c) -> p g c", g=JG),
                bg[:, :, 0:16], bg[:, :, 16:32], op=A.add)

        state = {}

        def ph1f(i):
            """load -> sigmoid -> swb"""
            rows = slice(i * P * TB, (i + 1) * P * TB)
            dview = logits_d[rows, :].rearrange("(j p) e -> p j e", p=P)

            scores = pool_sc.tile([P, W], f32, tag="scores")
            swb = pool_swb.tile([P, W], f32, tag="swb")

            step = TB // DMA_SPLIT
            for j in range(0, TB, step):
                sl = slice(j * E, (j + step) * E)
                nc.sync.dma_start(
                    scores[:, sl].rearrange("p (j e) -> p j e", j=step),
                    dview[:, j:j + step, :])
            nc.scalar.activation(scores[:], scores[:], AF.Sigmoid)
            nc.gpsimd.tensor_add(swb[:], scores[:], biasT[:])
            state[("f", i)] = (scores, swb)

        def ph1(i):
            """group scores -> goff -> swbm (issued late: swb has drained)"""
            scores, swb = state.pop(("f", i))
            swbm = pool_swbm.tile([P, W], f32, tag="swbm")

            if GS_MR:
                # m1 = per-group max; match_replace kills one occurrence of
                # each group's max (duplicate-safe); m2 = max of the rest.
                gpad = med.tile([P, 2048], f32, tag="gpad")
                m1g = gpad[:, 0:JG]
                m2g = gpad[:, 256:256 + JG]
                gs = gpad[:, 512:512 + JG]
                swbR = med.tile([P, W], f32, tag="swbR")
                nc.vector.tensor_reduce(
                    m1g, swb[:].rearrange("p (g e) -> p g e", g=JG),
                    axis=AX.X, op=A.max)
                for j in range(TB):
                    nc.vector.match_replace(
                        swbR[:, j * E:(j + 1) * E],
                        m1g[:, j * 8:(j + 1) * 8],
                        swb[:, j * E:(j + 1) * E], -BIG)
                nc.vector.tensor_reduce(
                    m2g, swbR[:].rearrange("p (g e) -> p g e", g=JG),
                    axis=AX.X, op=A.max)
                nc.vector.tensor_tensor(gs, m1g, m2g, op=A.add)

                gm8 = med.tile([P, TB * 8], f32, tag="gm8")
                for j in range(TB):
                    nc.vector.max(out=gm8[:, j * 8:(j + 1) * 8],
                                  in_=gs[:, j * G:(j + 1) * G])
                dpad = med.tile([P, 1024], f32, tag="dpad")
                goff = dpad[:, 512:512 + JG]
                for j in range(TB):
                    nc.vector.tensor_scalar(
                        goff[:, j * G:(j + 1) * G], gs[:, j * G:(j + 1) * G],
                        gm8[:, j * 8 + 3:j * 8 + 4], -BIG,
                        op0=A.is_lt, op1=A.mult)
                nc.gpsimd.tensor_tensor(
                    swbm[:].rearrange("p (j g e) -> p j g e", j=TB, g=G),
                    swb[:].rearrange("p (j g e) -> p j g e", j=TB, g=G),
                    goff.rearrange("p (j g) -> p j g", j=TB)
                    .to_broadcast([P, TB, G, EG]),
                    op=A.add)
                state[i] = (scores, swbm)
                return

            padd = nc.vector.tensor_tensor if TREE_DVE else nc.gpsimd.tensor_tensor
            m1 = med.tile([P, JG * 16], f32, tag="m1")
            m1v = m1[:].rearrange("p (g c) -> p g c", g=JG)
            g32 = swb[:].rearrange("p (g e) -> p g e", g=JG)
            a1, b1 = g32[:, :, 0:16], g32[:, :, 16:32]
            p1 = med.tile([P, JG * 16], f32, tag="p1")
            padd(p1[:].rearrange("p (g c) -> p g c", g=JG), a1, b1, op=A.add)
            nc.vector.tensor_tensor(m1v, a1, b1, op=A.max)
            a2, b2 = m1v[:, :, 0:8], m1v[:, :, 8:16]
            m2 = med.tile([P, JG * 8], f32, tag="m2")
            m2v = m2[:].rearrange("p (g c) -> p g c", g=JG)
            p2 = med.tile([P, JG * 8], f32, tag="p2")
            padd(p2[:].rearrange("p (g c) -> p g c", g=JG), a2, b2, op=A.add)
            nc.vector.tensor_tensor(m2v, a2, b2, op=A.max)
            a3, b3 = m2v[:, :, 0:4], m2v[:, :, 4:8]
            m3 = med.tile([P, JG * 4], f32, tag="m3")
            m3v = m3[:].rearrange("p (g c) -> p g c", g=JG)
            p3 = med.tile([P, JG * 4], f32, tag="p3")
            padd(p3[:].rearrange("p (g c) -> p g c", g=JG), a3, b3, op=A.add)
            nc.vector.tensor_tensor(m3v, a3, b3, op=A.max)
            a4, b4 = m3v[:, :, 0:2], m3v[:, :, 2:4]
            m4 = med.tile([P, JG * 2], f32, tag="m4")
            m4v = m4[:].rearrange("p (g c) -> p g c", g=JG)
            p4 = med.tile([P, JG * 2], f32, tag="p4")
            padd(p4[:].rearrange("p (g c) -> p g c", g=JG), a4, b4, op=A.add)
            nc.vector.tensor_tensor(m4v, a4, b4, op=A.max)

            gpad = med.tile([P, 2048], f32, tag="gpad")
            gs1 = gpad[:, 0:JG]
            gs2 = gpad[:, 256:256 + JG]
            gs3 = gpad[:, 512:512 + JG]
            gs4 = gpad[:, 768:768 + JG]
            gs = gpad[:, 1024:1024 + JG]
            padd(gs.rearrange("p (g c) -> p g c", c=1),
                 m4v[:, :, 0:1], m4v[:, :, 1:2], op=A.add)
            nc.vector.tensor_reduce(
                gs1, p1[:].rearrange("p (g c) -> p g c", g=JG),
                axis=AX.X, op=A.max)
            nc.vector.tensor_reduce(
                gs2, p2[:].rearrange("p (g c) -> p g c", g=JG),
                axis=AX.X, op=A.max)
            nc.vector.tensor_reduce(
                gs3, p3[:].rearrange("p (g c) -> p g c", g=JG),
                axis=AX.X, op=A.max)
            nc.vector.tensor_reduce(
                gs4, p4[:].rearrange("p (g c) -> p g c", g=JG),
                axis=AX.X, op=A.max)
            nc.vector.tensor_tensor(gs1, gs1, gs2, op=A.max)
            nc.vector.tensor_tensor(gs3, gs3, gs4, op=A.max)
            nc.vector.tensor_tensor(gs1, gs1, gs3, op=A.max)
            nc.vector.tensor_tensor(gs, gs, gs1, op=A.max)

            gm8 = med.tile([P, TB * 8], f32, tag="gm8")
            for j in range(TB):
                nc.vector.max(out=gm8[:, j * 8:(j + 1) * 8],
                              in_=gs[:, j * G:(j + 1) * G])
            dpad = med.tile([P, 1024], f32, tag="dpad")
            goff = dpad[:, 512:512 + JG]
            for j in range(TB):
                nc.vector.tensor_scalar(
                    goff[:, j * G:(j + 1) * G], gs[:, j * G:(j + 1) * G],
                    gm8[:, j * 8 + 3:j * 8 + 4], -BIG,
                    op0=A.is_lt, op1=A.mult)

            nc.gpsimd.tensor_tensor(
                swbm[:].rearrange("p (j g e) -> p j g e", j=TB, g=G),
                swb[:].rearrange("p (j g e) -> p j g e", j=TB, g=G),
                goff.rearrange("p (j g) -> p j g", j=TB)
                .to_broadcast([P, TB, G, EG]),
                op=A.add)
            state[i] = (scores, swbm)

        def ph2a(i):
            """v8b (nudge issued separately at end of iteration)"""
            scores, swbm = state[i]
            v8b = sel.tile([P, TB * 8], f32, tag="v8b")
            for j in range(TB):
                nc.vector.max(out=v8b[:, j * 8:(j + 1) * 8],
                              in_=swbm[:, j * E:(j + 1) * E])
            state[i] = (scores, swbm, v8b)

        def ph2n(i):
            """nudged threshold from v8b (late: v8b has drained)"""
            scores, swbm, v8b = state[i]
            c = 1.5 * 2.0 ** -23
            pad = sel.tile([P, 768], f32, tag="nudgepad")
            t8lo = pad[:, 0:TB]
            t8hi = pad[:, 256:256 + TB]
            nt8p = pad[:, 512:512 + TB]
            nc.vector.tensor_scalar(t8lo, v8b[:, 7::8], c - 1.0, None,
                                    op0=A.mult)
            nc.vector.tensor_scalar(t8hi, v8b[:, 7::8], -c - 1.0, None,
                                    op0=A.mult)
            nc.vector.tensor_tensor(nt8p, t8lo, t8hi, op=A.max)
            state[i] = (scores, swbm, nt8p)

        def ph2b(i):
            """sign select + s = scores * sgn"""
            scores, swbm, nt8p = state[i]
            for j in range(TB):
                nc.scalar.activation(
                    swbm[:, j * E:(j + 1) * E], swbm[:, j * E:(j + 1) * E],
                    AF.Sign, bias=nt8p[:, j:j + 1], scale=1.0)
            s = pool_s.tile([P, W], f32, tag="s")
            nc.gpsimd.tensor_tensor(s[:], scores[:], swbm[:], op=A.mult)
            state[i] = s

        def ph3(i):
            """final top-8 + indices"""
            s = state.pop(i)
            v8u = out.tile([P, TB * 8], f32, tag="v8u")
            idx8 = out.tile([P, TB * 8], mybir.dt.int32, tag="idx8")
            for j in range(TB):
                nc.vector.max(out=v8u[:, j * 8:(j + 1) * 8],
                              in_=s[:, j * E:(j + 1) * E])
            for j in range(TB):
                nc.vector.max_index(
                    out=idx8[:, j * 8:(j + 1) * 8].bitcast(u32),
                    in_max=v8u[:, j * 8:(j + 1) * 8],
                    in_values=s[:, j * E:(j + 1) * E])
            ssum = out.tile([P, TB], f32, tag="ssum")
            nc.vector.tensor_reduce(
                ssum[:], v8u[:].rearrange("p (j k) -> p j k", j=TB),
                axis=AX.X, op=A.add)
            state[("o", i)] = (v8u, idx8, ssum)

        def ph3n(i):
            """reciprocal chain (late: ssum has drained)"""
            v8u, idx8, ssum = state.pop(("o", i))
            ssum4 = out.tile([P, TB], f32, tag="ssum4")
            nc.vector.tensor_scalar(ssum4[:], ssum[:], 0.4, None, op0=A.mult)
            rec = out.tile([P, TB], f32, tag="rec")
            nc.vector.reciprocal(rec[:], ssum4[:])
            state[("o", i)] = (v8u, idx8, rec)

        def ph3b(i):
            v8u, idx8, rec = state.pop(("o", i))
            rows = slice(i * P * TB, (i + 1) * P * TB)
            vals8 = out.tile([P, TB * 8], f32, tag="vals8")
            nc.gpsimd.tensor_tensor(
                vals8[:].rearrange("p (j k) -> p j k", j=TB),
                v8u[:].rearrange("p (j k) -> p j k", j=TB),
                rec[:].rearrange("p (j o) -> p j o", o=1).to_broadcast([P, TB, 8]),
                op=A.mult)
            oi = idx_d[rows, :].rearrange("(j p) k -> p j k", p=P)
            ov = vals_d[rows, :].rearrange("(j p) k -> p j k", p=P)
            nc.sync.dma_start(oi, idx8[:].rearrange("p (j k) -> p j k", j=TB))
            nc.sync.dma_start(ov, vals8[:].rearrange("p (j k) -> p j k", j=TB))

        def live(j):
            return 0 <= j < n_groups

        for k in range(n_groups + 4):
            if live(k):
                ph1f(k)
            if live(k - 2):
                ph2b(k - 2)
            if live(k - 3):
                ph3(k - 3)
            if live(k - 4):
                ph3b(k - 4)
            if live(k - 1):
                ph2a(k - 1)
            if live(k):
                ph1(k)
            if live(k - 1):
                ph2n(k - 1)
            if live(k - 3):
                ph3n(k - 3)

    nc.compile()
    return nc


_NC_CACHE = {}


def _get_nc(n_tokens: int):
    if n_tokens not in _NC_CACHE:
        _NC_CACHE[n_tokens] = build_bass(n_tokens)
    return _NC_CACHE[n_tokens]


def _host_tiles(bias):
    biasb = np.ascontiguousarray(
        np.broadcast_to(np.tile(bias, TB)[None, :], (P, TB * E)).astype(np.float32))
    return biasb


def run_spmd(nc, logits, biasb, trace=False):
    from concourse import bass_utils

    n = logits.shape[0] // N_CORES
    in_maps = [
        {"logits": np.ascontiguousarray(logits[c * n:(c + 1) * n]),
         "biasb": biasb}
        for c in range(N_CORES)
    ]
    res = bass_utils.run_bass_kernel_spmd(nc, in_maps, list(range(N_CORES)),
                                          trace=trace)
    idx = np.concatenate([r["idx"] for r in res.results], axis=0)
    vals = np.concatenate([r["vals"] for r in res.results], axis=0)
    return (idx.astype(np.int32), vals.astype(np.float32)), res


def kernel(logits, e_score_correction_bias):
    logits = np.asarray(logits, dtype=np.float32)
    bias = np.asarray(e_score_correction_bias, dtype=np.float32)
    assert logits.shape == (T_FULL, E)
    biasb = _host_tiles(bias)
    nc = _get_nc(T_CORE)
    (idx, vals), _ = run_spmd(nc, logits, biasb)
    return idx, vals


# revision 3
# speedup vs baseline: 1.0287x; 1.0287x over previous
"""DeepSeek-V3 MoE routing kernel for Trainium2 (Bass/Tile), 8-core SPMD.

v2 redesign from measured per-op engine rates:
  ACT  0.83 ns/elem (+285 fixed)  - sigmoid, Sign-select
  Pool 2.13 ns/elem (+150)        - add/sub/mult ONLY (no max/compare!)
  DVE  1.07 ns/elem (+140)        - everything incl. max8/find_index8/reduce
  Pool TENSOR_SCALAR = 14 ns/elem - NEVER use (the old kernel's bottleneck)

Group top-2 via the exact identity t2(S) = max(pair-sum levels):
pair-sums on Pool (add), pair-maxes on DVE (native max), final grouped
max-reduce on DVE.  Group/top-8 masking additive (-1e30).  Selection by
ACT Sign with nudged threshold, final top-8 + indices via DVE
max8/match_value/find_index8 on true scores.

Software-pipelined phases per iteration k:
  ph1f(k)   load + sigmoid (ACT) + bias add (Pool)
  ph2b(k-2) Sign select (ACT) + s = scores*sgn (Pool)
  ph3(k-3)  max8/find_index8/ssum on s (DVE)
  ph3b(k-4) vals renorm (Pool) + output DMA
  ph2a(k-1) v8b max8 (DVE)
  ph1(k)    pair-tree group scores + goff + swbm  [late: swb drained]
  ph2n(k-1) nudged threshold                      [late: v8b drained]
  ph3n(k-3) reciprocal chain                      [late: ssum drained]
Late issuance matters: a consumer scheduled right after its producer pays
~2-3us of write-drain latency on this hardware; orders above hide it.
Deeper bufs on small tile pools avoid WAR semaphore stalls.
"""

import numpy as np

T_FULL = 131072
E = 256
G = 8
EG = 32
N_CORES = 8
T_CORE = T_FULL // N_CORES
P = 128
BIG = 1.0e30
TB = 8  # token-chunks (of P) per tile group
W = TB * E
JG = TB * G  # group-scores per partition row
DMA_SPLIT = 2  # input DMA issued in this many chunks
SEL_STT = False  # selection via fused DVE scalar_tensor_tensor (vs ACT Sign)
TREE_DVE = True  # pair-sums (levels 2+) on DVE instead of Pool
POOL_P1 = False  # level-1 pair-sum on Pool from scores + pairsum(bias) const
GS_MR = False  # group scores via reduce + match_replace + reduce (no ladder)


def build_bass(n_tokens: int):
    from contextlib import ExitStack

    import concourse.bacc as bacc
    import concourse.mybir as mybir
    import concourse.tile as tile

    f32 = mybir.dt.float32
    u32 = mybir.dt.uint32
    A = mybir.AluOpType
    AX = mybir.AxisListType
    AF = mybir.ActivationFunctionType

    assert n_tokens % (P * TB) == 0
    n_groups = n_tokens // (P * TB)

    nc = bacc.Bacc("TRN2", target_bir_lowering=False, debug=False)

    logits_d = nc.dram_tensor("logits", [n_tokens, E], f32, kind="ExternalInput").ap()
    biasb_d = nc.dram_tensor("biasb", [P, W], f32, kind="ExternalInput").ap()
    idx_d = nc.dram_tensor("idx", [n_tokens, 8], mybir.dt.int32, kind="ExternalOutput").ap()
    vals_d = nc.dram_tensor("vals", [n_tokens, 8], f32, kind="ExternalOutput").ap()

    with tile.TileContext(nc) as tc, ExitStack() as ctx:
        setup = ctx.enter_context(tc.tile_pool(name="setup", bufs=1))
        pool_sc = ctx.enter_context(tc.tile_pool(name="sc", bufs=4))
        pool_swb = ctx.enter_context(tc.tile_pool(name="swb", bufs=2))
        pool_swbm = ctx.enter_context(tc.tile_pool(name="swbm", bufs=4))
        pool_s = ctx.enter_context(tc.tile_pool(name="s", bufs=4))
        med = ctx.enter_context(tc.tile_pool(name="med", bufs=2))
        sel = ctx.enter_context(tc.tile_pool(name="sel", bufs=6))
        out = ctx.enter_context(tc.tile_pool(name="out", bufs=6))

        biasT = setup.tile([P, W], f32)
        nc.sync.dma_start(biasT[:], biasb_d)
        pbias = setup.tile([P, JG * 16], f32)
        if POOL_P1:
            bg = biasT[:].rearrange("p (g e) -> p g e", g=JG)
            nc.gpsimd.tensor_tensor(
                pbias[:].rearrange("p (g c) -> p g c", g=JG),
                bg[:, :, 0:16], bg[:, :, 16:32], op=A.add)

        state = {}

        def ph1f(i):
            """load -> sigmoid -> swb"""
            rows = slice(i * P * TB, (i + 1) * P * TB)
            dview = logits_d[rows, :].rearrange("(j p) e -> p j e", p=P)

            scores = pool_sc.tile([P, W], f32, tag="scores")
            swb = pool_swb.tile([P, W], f32, tag="swb")

            step = TB // DMA_SPLIT
            for j in range(0, TB, step):
                sl = slice(j * E, (j + step) * E)
                nc.sync.dma_start(
                    scores[:, sl].rearrange("p (j e) -> p j e", j=step),
                    dview[:, j:j + step, :])
            nc.scalar.activation(scores[:], scores[:], AF.Sigmoid)
            nc.gpsimd.tensor_add(swb[:], scores[:], biasT[:])
            state[("f", i)] = (scores, swb)

        def ph1(i):
            """group scores -> goff -> swbm (issued late: swb has drained)"""
            scores, swb = state.pop(("f", i))
            swbm = pool_swbm.tile([P, W], f32, tag="swbm")

            if GS_MR:
                # m1 = per-group max; match_replace kills one occurrence of
                # each group's max (duplicate-safe); m2 = max of the rest.
                gpad = med.tile([P, 2048], f32, tag="gpad")
                m1g = gpad[:, 0:JG]
                m2g = gpad[:, 256:256 + JG]
                gs = gpad[:, 512:512 + JG]
                swbR = med.tile([P, W], f32, tag="swbR")
                nc.vector.tensor_reduce(
                    m1g, swb[:].rearrange("p (g e) -> p g e", g=JG),
                    axis=AX.X, op=A.max)
                for j in range(TB):
                    nc.vector.match_replace(
                        swbR[:, j * E:(j + 1) * E],
                        m1g[:, j * 8:(j + 1) * 8],
                        swb[:, j * E:(j + 1) * E], -BIG)
                nc.vector.tensor_reduce(
                    m2g, swbR[:].rearrange("p (g e) -> p g e", g=JG),
                    axis=AX.X, op=A.max)
                nc.vector.tensor_tensor(gs, m1g, m2g, op=A.add)

                gm8 = med.tile([P, TB * 8], f32, tag="gm8")
                for j in range(TB):
                    nc.vector.max(out=gm8[:, j * 8:(j + 1) * 8],
                                  in_=gs[:, j * G:(j + 1) * G])
                dpad = med.tile([P, 1024], f32, tag="dpad")
                goff = dpad[:, 512:512 + JG]
                for j in range(TB):
                    nc.vector.tensor_scalar(
                        goff[:, j * G:(j + 1) * G], gs[:, j * G:(j + 1) * G],
                        gm8[:, j * 8 + 3:j * 8 + 4], -BIG,
                        op0=A.is_lt, op1=A.mult)
                nc.gpsimd.tensor_tensor(
                    swbm[:].rearrange("p (j g e) -> p j g e", j=TB, g=G),
                    swb[:].rearrange("p (j g e) -> p j g e", j=TB, g=G),
                    goff.rearrange("p (j g) -> p j g", j=TB)
                    .to_broadcast([P, TB, G, EG]),
                    op=A.add)
                state[i] = (scores, swbm)
                return

            padd = nc.vector.tensor_tensor if TREE_DVE else nc.gpsimd.tensor_tensor
            m1 = med.tile([P, JG * 16], f32, tag="m1")
            m1v = m1[:].rearrange("p (g c) -> p g c", g=JG)
            g32 = swb[:].rearrange("p (g e) -> p g e", g=JG)
            a1, b1 = g32[:, :, 0:16], g32[:, :, 16:32]
            p1 = med.tile([P, JG * 16], f32, tag="p1")
            padd(p1[:].rearrange("p (g c) -> p g c", g=JG), a1, b1, op=A.add)
            nc.vector.tensor_tensor(m1v, a1, b1, op=A.max)
            a2, b2 = m1v[:, :, 0:8], m1v[:, :, 8:16]
            m2 = med.tile([P, JG * 8], f32, tag="m2")
            m2v = m2[:].rearrange("p (g c) -> p g c", g=JG)
            p2 = med.tile([P, JG * 8], f32, tag="p2")
            padd(p2[:].rearrange("p (g c) -> p g c", g=JG), a2, b2, op=A.add)
            nc.vector.tensor_tensor(m2v, a2, b2, op=A.max)
            a3, b3 = m2v[:, :, 0:4], m2v[:, :, 4:8]
            m3 = med.tile([P, JG * 4], f32, tag="m3")
            m3v = m3[:].rearrange("p (g c) -> p g c", g=JG)
            p3 = med.tile([P, JG * 4], f32, tag="p3")
            padd(p3[:].rearrange("p (g c) -> p g c", g=JG), a3, b3, op=A.add)
            nc.vector.tensor_tensor(m3v, a3, b3, op=A.max)
            a4, b4 = m3v[:, :, 0:2], m3v[:, :, 2:4]
            m4 = med.tile([P, JG * 2], f32, tag="m4")
            m4v = m4[:].rearrange("p (g c) -> p g c", g=JG)
            p4 = med.tile([P, JG * 2], f32, tag="p4")
            padd(p4[:].rearrange("p (g c) -> p g c", g=JG), a4, b4, op=A.add)
            nc.vector.tensor_tensor(m4v, a4, b4, op=A.max)

            gpad = med.tile([P, 2048], f32, tag="gpad")
            gs1 = gpad[:, 0:JG]
            gs2 = gpad[:, 256:256 + JG]
            gs3 = gpad[:, 512:512 + JG]
            gs4 = gpad[:, 768:768 + JG]
            gs = gpad[:, 1024:1024 + JG]
            padd(gs.rearrange("p (g c) -> p g c", c=1),
                 m4v[:, :, 0:1], m4v[:, :, 1:2], op=A.add)
            nc.vector.tensor_reduce(
                gs1, p1[:].rearrange("p (g c) -> p g c", g=JG),
                axis=AX.X, op=A.max)
            nc.vector.tensor_reduce(
                gs2, p2[:].rearrange("p (g c) -> p g c", g=JG),
                axis=AX.X, op=A.max)
            nc.vector.tensor_reduce(
                gs3, p3[:].rearrange("p (g c) -> p g c", g=JG),
                axis=AX.X, op=A.max)
            nc.vector.tensor_reduce(
                gs4, p4[:].rearrange("p (g c) -> p g c", g=JG),
                axis=AX.X, op=A.max)
            nc.vector.tensor_tensor(gs1, gs1, gs2, op=A.max)
            nc.vector.tensor_tensor(gs3, gs3, gs4, op=A.max)
            nc.vector.tensor_tensor(gs1, gs1, gs3, op=A.max)
            nc.vector.tensor_tensor(gs, gs, gs1, op=A.max)

            gm8 = med.tile([P, TB * 8], f32, tag="gm8")
            for j in range(TB):
                nc.vector.max(out=gm8[:, j * 8:(j + 1) * 8],
                              in_=gs[:, j * G:(j + 1) * G])
            dpad = med.tile([P, 1024], f32, tag="dpad")
            goff = dpad[:, 512:512 + JG]
            for j in range(TB):
                nc.vector.tensor_scalar(
                    goff[:, j * G:(j + 1) * G], gs[:, j * G:(j + 1) * G],
                    gm8[:, j * 8 + 3:j * 8 + 4], -BIG,
                    op0=A.is_lt, op1=A.mult)

            nc.gpsimd.tensor_tensor(
                swbm[:].rearrange("p (j g e) -> p j g e", j=TB, g=G),
                swb[:].rearrange("p (j g e) -> p j g e", j=TB, g=G),
                goff.rearrange("p (j g) -> p j g", j=TB)
                .to_broadcast([P, TB, G, EG]),
                op=A.add)
            state[i] = (scores, swbm)

        def ph2a(i):
            """v8b (nudge issued separately at end of iteration)"""
            scores, swbm = state[i]
            v8b = sel.tile([P, TB * 8], f32, tag="v8b")
            for j in range(TB):
                nc.vector.max(out=v8b[:, j * 8:(j + 1) * 8],
                              in_=swbm[:, j * E:(j + 1) * E])
            state[i] = (scores, swbm, v8b)

        def ph2n(i):
            """nudged threshold from v8b (late: v8b has drained)"""
            scores, swbm, v8b = state[i]
            c = 1.5 * 2.0 ** -23
            pad = sel.tile([P, 768], f32, tag="nudgepad")
            t8lo = pad[:, 0:TB]
            t8hi = pad[:, 256:256 + TB]
            nt8p = pad[:, 512:512 + TB]
            nc.vector.tensor_scalar(t8lo, v8b[:, 7::8], c - 1.0, None,
                                    op0=A.mult)
            nc.vector.tensor_scalar(t8hi, v8b[:, 7::8], -c - 1.0, None,
                                    op0=A.mult)
            nc.vector.tensor_tensor(nt8p, t8lo, t8hi, op=A.max)
            state[i] = (scores, swbm, nt8p)

        def ph2b(i):
            """sign select + s = scores * sgn"""
            scores, swbm, nt8p = state[i]
            for j in range(TB):
                nc.scalar.activation(
                    swbm[:, j * E:(j + 1) * E], swbm[:, j * E:(j + 1) * E],
                    AF.Sign, bias=nt8p[:, j:j + 1], scale=1.0)
            s = pool_s.tile([P, W], f32, tag="s")
            nc.gpsimd.tensor_tensor(s[:], scores[:], swbm[:], op=A.mult)
            state[i] = s

        def ph3(i):
            """final top-8 + indices"""
            s = state.pop(i)
            v8u = out.tile([P, TB * 8], f32, tag="v8u")
            idx8 = out.tile([P, TB * 8], mybir.dt.int32, tag="idx8")
            for j in range(TB):
                nc.vector.max(out=v8u[:, j * 8:(j + 1) * 8],
                              in_=s[:, j * E:(j + 1) * E])
            for j in range(TB):
                nc.vector.max_index(
                    out=idx8[:, j * 8:(j + 1) * 8].bitcast(u32),
                    in_max=v8u[:, j * 8:(j + 1) * 8],
                    in_values=s[:, j * E:(j + 1) * E])
            ssum = out.tile([P, TB], f32, tag="ssum")
            nc.vector.tensor_reduce(
                ssum[:], v8u[:].rearrange("p (j k) -> p j k", j=TB),
                axis=AX.X, op=A.add)
            state[("o", i)] = (v8u, idx8, ssum)

        def ph3n(i):
            """reciprocal chain (late: ssum has drained)"""
            v8u, idx8, ssum = state.pop(("o", i))
            ssum4 = out.tile([P, TB], f32, tag="ssum4")
            nc.vector.tensor_scalar(ssum4[:], ssum[:], 0.4, None, op0=A.mult)
            rec = out.tile([P, TB], f32, tag="rec")
            nc.vector.reciprocal(rec[:], ssum4[:])
            state[("o", i)] = (v8u, idx8, rec)

        def ph3b(i):
            v8u, idx8, rec = state.pop(("o", i))
            rows = slice(i * P * TB, (i + 1) * P * TB)
            vals8 = out.tile([P, TB * 8], f32, tag="vals8")
            nc.gpsimd.tensor_tensor(
                vals8[:].rearrange("p (j k) -> p j k", j=TB),
                v8u[:].rearrange("p (j k) -> p j k", j=TB),
                rec[:].rearrange("p (j o) -> p j o", o=1).to_broadcast([P, TB, 8]),
                op=A.mult)
            oi = idx_d[rows, :].rearrange("(j p) k -> p j k", p=P)
            ov = vals_d[rows, :].rearrange("(j p) k -> p j k", p=P)
            nc.sync.dma_start(oi, idx8[:].rearrange("p (j k) -> p j k", j=TB))
            nc.sync.dma_start(ov, vals8[:].rearrange("p (j k) -> p j k", j=TB))

        def live(j):
            return 0 <= j < n_groups

        for k in range(n_groups + 4):
            if live(k):
                ph1f(k)
            if live(k - 2):
                ph2b(k - 2)
            if live(k - 3):
                ph3(k - 3)
            if live(k - 4):
                ph3b(k - 4)
            if live(k - 1):
                ph2a(k - 1)
            if live(k):
                ph1(k)
            if live(k - 1):
                ph2n(k - 1)
            if live(k - 3):
                ph3n(k - 3)

    nc.compile()
    return nc


_NC_CACHE = {}


def _get_nc(n_tokens: int):
    if n_tokens not in _NC_CACHE:
        _NC_CACHE[n_tokens] = build_bass(n_tokens)
    return _NC_CACHE[n_tokens]


def _host_tiles(bias):
    biasb = np.ascontiguousarray(
        np.broadcast_to(np.tile(bias, TB)[None, :], (P, TB * E)).astype(np.float32))
    return biasb


def run_spmd(nc, logits, biasb, trace=False):
    from concourse import bass_utils

    n = logits.shape[0] // N_CORES
    in_maps = [
        {"logits": np.ascontiguousarray(logits[c * n:(c + 1) * n]),
         "biasb": biasb}
        for c in range(N_CORES)
    ]
    res = bass_utils.run_bass_kernel_spmd(nc, in_maps, list(range(N_CORES)),
                                          trace=trace)
    idx = np.concatenate([r["idx"] for r in res.results], axis=0)
    vals = np.concatenate([r["vals"] for r in res.results], axis=0)
    return (idx.astype(np.int32), vals.astype(np.float32)), res


def kernel(logits, e_score_correction_bias):
    logits = np.asarray(logits, dtype=np.float32)
    bias = np.asarray(e_score_correction_bias, dtype=np.float32)
    assert logits.shape == (T_FULL, E)
    biasb = _host_tiles(bias)
    nc = _get_nc(T_CORE)
    (idx, vals), _ = run_spmd(nc, logits, biasb)
    return idx, vals


# revision 4
# speedup vs baseline: 1.0354x; 1.0065x over previous
"""DeepSeek-V3 MoE routing kernel for Trainium2 (Bass/Tile), 8-core SPMD.

v2 redesign from measured per-op engine rates:
  ACT  0.83 ns/elem (+285 fixed)  - sigmoid, Sign-select
  Pool 2.13 ns/elem (+150)        - add/sub/mult ONLY (no max/compare!)
  DVE  1.07 ns/elem (+140)        - everything incl. max8/find_index8/reduce
  Pool TENSOR_SCALAR = 14 ns/elem - NEVER use (the old kernel's bottleneck)

Group top-2 via the exact identity t2(S) = max(pair-sum levels):
pair-sums on Pool (add), pair-maxes on DVE (native max), final grouped
max-reduce on DVE.  Group/top-8 masking additive (-1e30).  Selection by
ACT Sign with nudged threshold, final top-8 + indices via DVE
max8/match_value/find_index8 on true scores.

Software-pipelined phases per iteration k:
  ph1f(k)   load + sigmoid (ACT) + bias add (Pool)
  ph2b(k-2) Sign select (ACT) + s = scores*sgn (Pool)
  ph3(k-3)  max8/find_index8/ssum on s (DVE)
  ph3b(k-4) vals renorm (Pool) + output DMA
  ph2a(k-1) v8b max8 (DVE)
  ph1(k)    pair-tree group scores + goff + swbm  [late: swb drained]
  ph2n(k-1) nudged threshold                      [late: v8b drained]
  ph3n(k-3) reciprocal chain                      [late: ssum drained]
Late issuance matters: a consumer scheduled right after its producer pays
~2-3us of write-drain latency on this hardware; orders above hide it.
Deeper bufs on small tile pools avoid WAR semaphore stalls.
"""

import numpy as np

T_FULL = 131072
E = 256
G = 8
EG = 32
N_CORES = 8
T_CORE = T_FULL // N_CORES
P = 128
BIG = 1.0e30
TB = 8  # token-chunks (of P) per tile group
W = TB * E
JG = TB * G  # group-scores per partition row
DMA_SPLIT = 2  # input DMA issued in this many chunks
SEL_STT = False  # selection via fused DVE scalar_tensor_tensor (vs ACT Sign)
TREE_DVE = True  # pair-sums (levels 2+) on DVE instead of Pool
POOL_P1 = False  # level-1 pair-sum on Pool from scores + pairsum(bias) const
GS_MR = False  # group scores via reduce + match_replace + reduce (no ladder)


def build_bass(n_tokens: int):
    from contextlib import ExitStack

    import concourse.bacc as bacc
    import concourse.mybir as mybir
    import concourse.tile as tile

    f32 = mybir.dt.float32
    u32 = mybir.dt.uint32
    A = mybir.AluOpType
    AX = mybir.AxisListType
    AF = mybir.ActivationFunctionType

    assert n_tokens % (P * TB) == 0
    n_groups = n_tokens // (P * TB)

    nc = bacc.Bacc("TRN2", target_bir_lowering=False, debug=False)

    logits_d = nc.dram_tensor("logits", [n_tokens, E], f32, kind="ExternalInput").ap()
    biasb_d = nc.dram_tensor("biasb", [P, W], f32, kind="ExternalInput").ap()
    idx_d = nc.dram_tensor("idx", [n_tokens, 8], mybir.dt.int32, kind="ExternalOutput").ap()
    vals_d = nc.dram_tensor("vals", [n_tokens, 8], f32, kind="ExternalOutput").ap()

    with tile.TileContext(nc) as tc, ExitStack() as ctx:
        setup = ctx.enter_context(tc.tile_pool(name="setup", bufs=1))
        pool_sc = ctx.enter_context(tc.tile_pool(name="sc", bufs=4))
        pool_swb = ctx.enter_context(tc.tile_pool(name="swb", bufs=2))
        pool_swbm = ctx.enter_context(tc.tile_pool(name="swbm", bufs=4))
        pool_s = ctx.enter_context(tc.tile_pool(name="s", bufs=4))
        med = ctx.enter_context(tc.tile_pool(name="med", bufs=2))
        sel = ctx.enter_context(tc.tile_pool(name="sel", bufs=6))
        out = ctx.enter_context(tc.tile_pool(name="out", bufs=6))

        biasT = setup.tile([P, W], f32)
        nc.sync.dma_start(biasT[:], biasb_d)
        pbias = setup.tile([P, JG * 16], f32)
        if POOL_P1:
            bg = biasT[:].rearrange("p (g e) -> p g e", g=JG)
            nc.gpsimd.tensor_tensor(
                pbias[:].rearrange("p (g c) -> p g c", g=JG),
                bg[:, :, 0:16], bg[:, :, 16:32], op=A.add)

        state = {}

        def ph1f(i):
            """load -> sigmoid -> swb"""
            rows = slice(i * P * TB, (i + 1) * P * TB)
            dview = logits_d[rows, :].rearrange("(j p) e -> p j e", p=P)

            scores = pool_sc.tile([P, W], f32, tag="scores")
            swb = pool_swb.tile([P, W], f32, tag="swb")

            step = TB // DMA_SPLIT
            for j in range(0, TB, step):
                sl = slice(j * E, (j + step) * E)
                nc.sync.dma_start(
                    scores[:, sl].rearrange("p (j e) -> p j e", j=step),
                    dview[:, j:j + step, :])
            nc.scalar.activation(scores[:], scores[:], AF.Sigmoid)
            nc.gpsimd.tensor_add(swb[:], scores[:], biasT[:])
            state[("f", i)] = (scores, swb)

        def ph1(i):
            """group scores -> goff -> swbm (issued late: swb has drained)"""
            scores, swb = state.pop(("f", i))
            swbm = pool_swbm.tile([P, W], f32, tag="swbm")

            if GS_MR:
                # m1 = per-group max; match_replace kills one occurrence of
                # each group's max (duplicate-safe); m2 = max of the rest.
                gpad = med.tile([P, 2048], f32, tag="gpad")
                m1g = gpad[:, 0:JG]
                m2g = gpad[:, 256:256 + JG]
                gs = gpad[:, 512:512 + JG]
                swbR = med.tile([P, W], f32, tag="swbR")
                nc.vector.tensor_reduce(
                    m1g, swb[:].rearrange("p (g e) -> p g e", g=JG),
                    axis=AX.X, op=A.max)
                for j in range(TB):
                    nc.vector.match_replace(
                        swbR[:, j * E:(j + 1) * E],
                        m1g[:, j * 8:(j + 1) * 8],
                        swb[:, j * E:(j + 1) * E], -BIG)
                nc.vector.tensor_reduce(
                    m2g, swbR[:].rearrange("p (g e) -> p g e", g=JG),
                    axis=AX.X, op=A.max)
                nc.vector.tensor_tensor(gs, m1g, m2g, op=A.add)

                gm8 = med.tile([P, TB * 8], f32, tag="gm8")
                for j in range(TB):
                    nc.vector.max(out=gm8[:, j * 8:(j + 1) * 8],
                                  in_=gs[:, j * G:(j + 1) * G])
                dpad = med.tile([P, 1024], f32, tag="dpad")
                goff = dpad[:, 512:512 + JG]
                for j in range(TB):
                    nc.vector.tensor_scalar(
                        goff[:, j * G:(j + 1) * G], gs[:, j * G:(j + 1) * G],
                        gm8[:, j * 8 + 3:j * 8 + 4], -BIG,
                        op0=A.is_lt, op1=A.mult)
                nc.gpsimd.tensor_tensor(
                    swbm[:].rearrange("p (j g e) -> p j g e", j=TB, g=G),
                    swb[:].rearrange("p (j g e) -> p j g e", j=TB, g=G),
                    goff.rearrange("p (j g) -> p j g", j=TB)
                    .to_broadcast([P, TB, G, EG]),
                    op=A.add)
                state[i] = (scores, swbm)
                return

            padd = nc.vector.tensor_tensor if TREE_DVE else nc.gpsimd.tensor_tensor
            m1 = med.tile([P, JG * 16], f32, tag="m1")
            m1v = m1[:].rearrange("p (g c) -> p g c", g=JG)
            g32 = swb[:].rearrange("p (g e) -> p g e", g=JG)
            a1, b1 = g32[:, :, 0:16], g32[:, :, 16:32]
            p1 = med.tile([P, JG * 16], f32, tag="p1")
            padd(p1[:].rearrange("p (g c) -> p g c", g=JG), a1, b1, op=A.add)
            nc.vector.tensor_tensor(m1v, a1, b1, op=A.max)
            a2, b2 = m1v[:, :, 0:8], m1v[:, :, 8:16]
            m2 = med.tile([P, JG * 8], f32, tag="m2")
            m2v = m2[:].rearrange("p (g c) -> p g c", g=JG)
            p2 = med.tile([P, JG * 8], f32, tag="p2")
            padd(p2[:].rearrange("p (g c) -> p g c", g=JG), a2, b2, op=A.add)
            nc.vector.tensor_tensor(m2v, a2, b2, op=A.max)
            a3, b3 = m2v[:, :, 0:4], m2v[:, :, 4:8]
            m3 = med.tile([P, JG * 4], f32, tag="m3")
            m3v = m3[:].rearrange("p (g c) -> p g c", g=JG)
            p3 = med.tile([P, JG * 4], f32, tag="p3")
            padd(p3[:].rearrange("p (g c) -> p g c", g=JG), a3, b3, op=A.add)
            nc.vector.tensor_tensor(m3v, a3, b3, op=A.max)
            a4, b4 = m3v[:, :, 0:2], m3v[:, :, 2:4]
            m4 = med.tile([P, JG * 2], f32, tag="m4")
            m4v = m4[:].rearrange("p (g c) -> p g c", g=JG)
            p4 = med.tile([P, JG * 2], f32, tag="p4")
            padd(p4[:].rearrange("p (g c) -> p g c", g=JG), a4, b4, op=A.add)
            nc.vector.tensor_tensor(m4v, a4, b4, op=A.max)

            gpad = med.tile([P, 2048], f32, tag="gpad")
            gs1 = gpad[:, 0:JG]
            gs2 = gpad[:, 64:64 + JG]
            gs3 = gpad[:, 128:128 + JG]
            gs4 = gpad[:, 192:192 + JG]
            gs5 = gpad[:, 256:256 + JG]
            gs = gpad[:, 1024:1024 + JG]
            padd(gs5.rearrange("p (g c) -> p g c", c=1),
                 m4v[:, :, 0:1], m4v[:, :, 1:2], op=A.add)
            nc.vector.tensor_reduce(
                gs1, p1[:].rearrange("p (g c) -> p g c", g=JG),
                axis=AX.X, op=A.max)
            nc.vector.tensor_reduce(
                gs2, p2[:].rearrange("p (g c) -> p g c", g=JG),
                axis=AX.X, op=A.max)
            nc.vector.tensor_reduce(
                gs3, p3[:].rearrange("p (g c) -> p g c", g=JG),
                axis=AX.X, op=A.max)
            nc.vector.tensor_reduce(
                gs4, p4[:].rearrange("p (g c) -> p g c", g=JG),
                axis=AX.X, op=A.max)
            nc.vector.tensor_reduce(
                gs, gpad[:, 0:320].rearrange("p (c g) -> p g c", c=5),
                axis=AX.X, op=A.max)

            gm8 = med.tile([P, TB * 8], f32, tag="gm8")
            for j in range(TB):
                nc.vector.max(out=gm8[:, j * 8:(j + 1) * 8],
                              in_=gs[:, j * G:(j + 1) * G])
            dpad = med.tile([P, 1024], f32, tag="dpad")
            d = dpad[:, 0:JG]
            goff = dpad[:, 512:512 + JG]
            nc.vector.tensor_tensor(
                d.rearrange("p (j g) -> p j g", j=TB),
                gs.rearrange("p (j g) -> p j g", j=TB),
                gm8[:, 3::8].rearrange("p (j o) -> p j o", o=1)
                .to_broadcast([P, TB, G]),
                op=A.is_lt)
            nc.vector.tensor_scalar(goff, d, -BIG, None, op0=A.mult)

            nc.gpsimd.tensor_tensor(
                swbm[:].rearrange("p (j g e) -> p j g e", j=TB, g=G),
                swb[:].rearrange("p (j g e) -> p j g e", j=TB, g=G),
                goff.rearrange("p (j g) -> p j g", j=TB)
                .to_broadcast([P, TB, G, EG]),
                op=A.add)
            state[i] = (scores, swbm)

        def ph2a(i):
            """v8b (nudge issued separately at end of iteration)"""
            scores, swbm = state[i]
            v8b = sel.tile([P, TB * 8], f32, tag="v8b")
            for j in range(TB):
                nc.vector.max(out=v8b[:, j * 8:(j + 1) * 8],
                              in_=swbm[:, j * E:(j + 1) * E])
            state[i] = (scores, swbm, v8b)

        def ph2n(i):
            """nudged threshold from v8b (late: v8b has drained)"""
            scores, swbm, v8b = state[i]
            c = 1.5 * 2.0 ** -23
            pad = sel.tile([P, 768], f32, tag="nudgepad")
            t8lo = pad[:, 0:TB]
            t8hi = pad[:, 256:256 + TB]
            nt8p = pad[:, 512:512 + TB]
            nc.vector.tensor_scalar(t8lo, v8b[:, 7::8], c - 1.0, None,
                                    op0=A.mult)
            nc.vector.tensor_scalar(t8hi, v8b[:, 7::8], -c - 1.0, None,
                                    op0=A.mult)
            nc.vector.tensor_tensor(nt8p, t8lo, t8hi, op=A.max)
            state[i] = (scores, swbm, nt8p)

        def ph2b(i):
            """sign select + s = scores * sgn"""
            scores, swbm, nt8p = state[i]
            for j in range(TB):
                nc.scalar.activation(
                    swbm[:, j * E:(j + 1) * E], swbm[:, j * E:(j + 1) * E],
                    AF.Sign, bias=nt8p[:, j:j + 1], scale=1.0)
            s = pool_s.tile([P, W], f32, tag="s")
            nc.gpsimd.tensor_tensor(s[:], scores[:], swbm[:], op=A.mult)
            state[i] = s

        def ph3(i):
            """final top-8 + indices"""
            s = state.pop(i)
            v8u = out.tile([P, TB * 8], f32, tag="v8u")
            idx8 = out.tile([P, TB * 8], mybir.dt.int32, tag="idx8")
            for j in range(TB):
                nc.vector.max(out=v8u[:, j * 8:(j + 1) * 8],
                              in_=s[:, j * E:(j + 1) * E])
            for j in range(TB):
                nc.vector.max_index(
                    out=idx8[:, j * 8:(j + 1) * 8].bitcast(u32),
                    in_max=v8u[:, j * 8:(j + 1) * 8],
                    in_values=s[:, j * E:(j + 1) * E])
            ssum = out.tile([P, TB], f32, tag="ssum")
            nc.vector.tensor_reduce(
                ssum[:], v8u[:].rearrange("p (j k) -> p j k", j=TB),
                axis=AX.X, op=A.add)
            state[("o", i)] = (v8u, idx8, ssum)

        def ph3n(i):
            """reciprocal chain (late: ssum has drained)"""
            v8u, idx8, ssum = state.pop(("o", i))
            ssum4 = out.tile([P, TB], f32, tag="ssum4")
            nc.vector.tensor_scalar(ssum4[:], ssum[:], 0.4, None, op0=A.mult)
            rec = out.tile([P, TB], f32, tag="rec")
            nc.vector.reciprocal(rec[:], ssum4[:])
            state[("o", i)] = (v8u, idx8, rec)

        def ph3b(i):
            v8u, idx8, rec = state.pop(("o", i))
            rows = slice(i * P * TB, (i + 1) * P * TB)
            vals8 = out.tile([P, TB * 8], f32, tag="vals8")
            nc.gpsimd.tensor_tensor(
                vals8[:].rearrange("p (j k) -> p j k", j=TB),
                v8u[:].rearrange("p (j k) -> p j k", j=TB),
                rec[:].rearrange("p (j o) -> p j o", o=1).to_broadcast([P, TB, 8]),
                op=A.mult)
            oi = idx_d[rows, :].rearrange("(j p) k -> p j k", p=P)
            ov = vals_d[rows, :].rearrange("(j p) k -> p j k", p=P)
            nc.sync.dma_start(oi, idx8[:].rearrange("p (j k) -> p j k", j=TB))
            nc.sync.dma_start(ov, vals8[:].rearrange("p (j k) -> p j k", j=TB))

        def live(j):
            return 0 <= j < n_groups

        for k in range(n_groups + 4):
            if live(k):
                ph1f(k)
            if live(k - 2):
                ph2b(k - 2)
            if live(k - 3):
                ph3(k - 3)
            if live(k - 4):
                ph3b(k - 4)
            if live(k - 1):
                ph2a(k - 1)
            if live(k):
                ph1(k)
            if live(k - 1):
                ph2n(k - 1)
            if live(k - 3):
                ph3n(k - 3)

    nc.compile()
    return nc


_NC_CACHE = {}


def _get_nc(n_tokens: int):
    if n_tokens not in _NC_CACHE:
        _NC_CACHE[n_tokens] = build_bass(n_tokens)
    return _NC_CACHE[n_tokens]


def _host_tiles(bias):
    biasb = np.ascontiguousarray(
        np.broadcast_to(np.tile(bias, TB)[None, :], (P, TB * E)).astype(np.float32))
    return biasb


def run_spmd(nc, logits, biasb, trace=False):
    from concourse import bass_utils

    n = logits.shape[0] // N_CORES
    in_maps = [
        {"logits": np.ascontiguousarray(logits[c * n:(c + 1) * n]),
         "biasb": biasb}
        for c in range(N_CORES)
    ]
    res = bass_utils.run_bass_kernel_spmd(nc, in_maps, list(range(N_CORES)),
                                          trace=trace)
    idx = np.concatenate([r["idx"] for r in res.results], axis=0)
    vals = np.concatenate([r["vals"] for r in res.results], axis=0)
    return (idx.astype(np.int32), vals.astype(np.float32)), res


def kernel(logits, e_score_correction_bias):
    logits = np.asarray(logits, dtype=np.float32)
    bias = np.asarray(e_score_correction_bias, dtype=np.float32)
    assert logits.shape == (T_FULL, E)
    biasb = _host_tiles(bias)
    nc = _get_nc(T_CORE)
    (idx, vals), _ = run_spmd(nc, logits, biasb)
    return idx, vals


# revision 5
# speedup vs baseline: 1.0552x; 1.0191x over previous
"""DeepSeek-V3 MoE routing kernel for Trainium2 (Bass/Tile), 8-core SPMD.

v2 redesign from measured per-op engine rates:
  ACT  0.83 ns/elem (+285 fixed)  - sigmoid, Sign-select
  Pool 2.13 ns/elem (+150)        - add/sub/mult ONLY (no max/compare!)
  DVE  1.07 ns/elem (+140)        - everything incl. max8/find_index8/reduce
  Pool TENSOR_SCALAR = 14 ns/elem - NEVER use (the old kernel's bottleneck)

Group top-2 via the exact identity t2(S) = max(pair-sum levels):
pair-sums on Pool (add), pair-maxes on DVE (native max), final grouped
max-reduce on DVE.  Group/top-8 masking additive (-1e30).  Selection by
ACT Sign with nudged threshold, final top-8 + indices via DVE
max8/match_value/find_index8 on true scores.

Software-pipelined phases per iteration k:
  ph1f(k)   load + sigmoid (ACT) + bias add (Pool)
  ph2b(k-2) Sign select (ACT) + s = scores*sgn (Pool)
  ph3(k-3)  max8/find_index8/ssum on s (DVE)
  ph3b(k-4) vals renorm (Pool) + output DMA
  ph2a(k-1) v8b max8 (DVE)
  ph1(k)    pair-tree group scores + goff + swbm  [late: swb drained]
  ph2n(k-1) nudged threshold                      [late: v8b drained]
  ph3n(k-3) reciprocal chain                      [late: ssum drained]
Late issuance matters: a consumer scheduled right after its producer pays
~2-3us of write-drain latency on this hardware; orders above hide it.
Deeper bufs on small tile pools avoid WAR semaphore stalls.
"""

import numpy as np

T_FULL = 131072
E = 256
G = 8
EG = 32
N_CORES = 8
T_CORE = T_FULL // N_CORES
P = 128
BIG = 1.0e30
TB = 8  # token-chunks (of P) per tile group
W = TB * E
JG = TB * G  # group-scores per partition row
DMA_SPLIT = 1  # input DMA issued in this many chunks
SEL_STT = False  # selection via fused DVE scalar_tensor_tensor (vs ACT Sign)
TREE_DVE = True  # pair-sums (levels 2+) on DVE instead of Pool
POOL_P1 = False  # level-1 pair-sum on Pool from scores + pairsum(bias) const
GS_MR = False  # group scores via reduce + match_replace + reduce (no ladder)


def build_bass(n_tokens: int):
    from contextlib import ExitStack

    import concourse.bacc as bacc
    import concourse.mybir as mybir
    import concourse.tile as tile

    f32 = mybir.dt.float32
    u32 = mybir.dt.uint32
    A = mybir.AluOpType
    AX = mybir.AxisListType
    AF = mybir.ActivationFunctionType

    assert n_tokens % (P * TB) == 0
    n_groups = n_tokens // (P * TB)

    nc = bacc.Bacc("TRN2", target_bir_lowering=False, debug=False)

    logits_d = nc.dram_tensor("logits", [n_tokens, E], f32, kind="ExternalInput").ap()
    biasb_d = nc.dram_tensor("biasb", [P, W], f32, kind="ExternalInput").ap()
    idx_d = nc.dram_tensor("idx", [n_tokens, 8], mybir.dt.int32, kind="ExternalOutput").ap()
    vals_d = nc.dram_tensor("vals", [n_tokens, 8], f32, kind="ExternalOutput").ap()

    with tile.TileContext(nc) as tc, ExitStack() as ctx:
        setup = ctx.enter_context(tc.tile_pool(name="setup", bufs=1))
        pool_sc = ctx.enter_context(tc.tile_pool(name="sc", bufs=4))
        pool_swb = ctx.enter_context(tc.tile_pool(name="swb", bufs=2))
        pool_swbm = ctx.enter_context(tc.tile_pool(name="swbm", bufs=4))
        pool_s = ctx.enter_context(tc.tile_pool(name="s", bufs=4))
        med = ctx.enter_context(tc.tile_pool(name="med", bufs=2))
        sel = ctx.enter_context(tc.tile_pool(name="sel", bufs=6))
        out = ctx.enter_context(tc.tile_pool(name="out", bufs=6))

        biasT = setup.tile([P, W], f32)
        nc.sync.dma_start(biasT[:], biasb_d)
        pbias = setup.tile([P, JG * 16], f32)
        if POOL_P1:
            bg = biasT[:].rearrange("p (g e) -> p g e", g=JG)
            nc.gpsimd.tensor_tensor(
                pbias[:].rearrange("p (g c) -> p g c", g=JG),
                bg[:, :, 0:16], bg[:, :, 16:32], op=A.add)

        state = {}

        def ph1f(i):
            """load -> sigmoid -> swb"""
            rows = slice(i * P * TB, (i + 1) * P * TB)
            dview = logits_d[rows, :].rearrange("(j p) e -> p j e", p=P)

            scores = pool_sc.tile([P, W], f32, tag="scores")
            swb = pool_swb.tile([P, W], f32, tag="swb")

            step = TB // DMA_SPLIT
            for j in range(0, TB, step):
                sl = slice(j * E, (j + step) * E)
                nc.sync.dma_start(
                    scores[:, sl].rearrange("p (j e) -> p j e", j=step),
                    dview[:, j:j + step, :])
            nc.scalar.activation(scores[:], scores[:], AF.Sigmoid)
            nc.gpsimd.tensor_add(swb[:], scores[:], biasT[:])
            state[("f", i)] = (scores, swb)

        def ph1(i):
            """group scores -> goff -> swbm (issued late: swb has drained)"""
            scores, swb = state.pop(("f", i))
            swbm = pool_swbm.tile([P, W], f32, tag="swbm")

            if GS_MR:
                # m1 = per-group max; match_replace kills one occurrence of
                # each group's max (duplicate-safe); m2 = max of the rest.
                gpad = med.tile([P, 2048], f32, tag="gpad")
                m1g = gpad[:, 0:JG]
                m2g = gpad[:, 256:256 + JG]
                gs = gpad[:, 512:512 + JG]
                swbR = med.tile([P, W], f32, tag="swbR")
                nc.vector.tensor_reduce(
                    m1g, swb[:].rearrange("p (g e) -> p g e", g=JG),
                    axis=AX.X, op=A.max)
                for j in range(TB):
                    nc.vector.match_replace(
                        swbR[:, j * E:(j + 1) * E],
                        m1g[:, j * 8:(j + 1) * 8],
                        swb[:, j * E:(j + 1) * E], -BIG)
                nc.vector.tensor_reduce(
                    m2g, swbR[:].rearrange("p (g e) -> p g e", g=JG),
                    axis=AX.X, op=A.max)
                nc.vector.tensor_tensor(gs, m1g, m2g, op=A.add)

                gm8 = med.tile([P, TB * 8], f32, tag="gm8")
                for j in range(TB):
                    nc.vector.max(out=gm8[:, j * 8:(j + 1) * 8],
                                  in_=gs[:, j * G:(j + 1) * G])
                dpad = med.tile([P, 1024], f32, tag="dpad")
                goff = dpad[:, 512:512 + JG]
                for j in range(TB):
                    nc.vector.tensor_scalar(
                        goff[:, j * G:(j + 1) * G], gs[:, j * G:(j + 1) * G],
                        gm8[:, j * 8 + 3:j * 8 + 4], -BIG,
                        op0=A.is_lt, op1=A.mult)
                nc.gpsimd.tensor_tensor(
                    swbm[:].rearrange("p (j g e) -> p j g e", j=TB, g=G),
                    swb[:].rearrange("p (j g e) -> p j g e", j=TB, g=G),
                    goff.rearrange("p (j g) -> p j g", j=TB)
                    .to_broadcast([P, TB, G, EG]),
                    op=A.add)
                state[i] = (scores, swbm)
                return

            padd = nc.vector.tensor_tensor if TREE_DVE else nc.gpsimd.tensor_tensor
            m1 = med.tile([P, JG * 16], f32, tag="m1")
            m1v = m1[:].rearrange("p (g c) -> p g c", g=JG)
            g32 = swb[:].rearrange("p (g e) -> p g e", g=JG)
            a1, b1 = g32[:, :, 0:16], g32[:, :, 16:32]
            p1 = med.tile([P, JG * 16], f32, tag="p1")
            padd(p1[:].rearrange("p (g c) -> p g c", g=JG), a1, b1, op=A.add)
            nc.vector.tensor_tensor(m1v, a1, b1, op=A.max)
            a2, b2 = m1v[:, :, 0:8], m1v[:, :, 8:16]
            m2 = med.tile([P, JG * 8], f32, tag="m2")
            m2v = m2[:].rearrange("p (g c) -> p g c", g=JG)
            p2 = med.tile([P, JG * 8], f32, tag="p2")
            padd(p2[:].rearrange("p (g c) -> p g c", g=JG), a2, b2, op=A.add)
            nc.vector.tensor_tensor(m2v, a2, b2, op=A.max)
            a3, b3 = m2v[:, :, 0:4], m2v[:, :, 4:8]
            m3 = med.tile([P, JG * 4], f32, tag="m3")
            m3v = m3[:].rearrange("p (g c) -> p g c", g=JG)
            p3 = med.tile([P, JG * 4], f32, tag="p3")
            padd(p3[:].rearrange("p (g c) -> p g c", g=JG), a3, b3, op=A.add)
            nc.vector.tensor_tensor(m3v, a3, b3, op=A.max)
            a4, b4 = m3v[:, :, 0:2], m3v[:, :, 2:4]
            m4 = med.tile([P, JG * 2], f32, tag="m4")
            m4v = m4[:].rearrange("p (g c) -> p g c", g=JG)
            p4 = med.tile([P, JG * 2], f32, tag="p4")
            padd(p4[:].rearrange("p (g c) -> p g c", g=JG), a4, b4, op=A.add)
            nc.vector.tensor_tensor(m4v, a4, b4, op=A.max)

            gpad = med.tile([P, 2048], f32, tag="gpad")
            gs1 = gpad[:, 0:JG]
            gs2 = gpad[:, 64:64 + JG]
            gs3 = gpad[:, 128:128 + JG]
            gs4 = gpad[:, 192:192 + JG]
            gs5 = gpad[:, 256:256 + JG]
            gs = gpad[:, 1024:1024 + JG]
            padd(gs5.rearrange("p (g c) -> p g c", c=1),
                 m4v[:, :, 0:1], m4v[:, :, 1:2], op=A.add)
            nc.vector.tensor_reduce(
                gs1, p1[:].rearrange("p (g c) -> p g c", g=JG),
                axis=AX.X, op=A.max)
            nc.vector.tensor_reduce(
                gs2, p2[:].rearrange("p (g c) -> p g c", g=JG),
                axis=AX.X, op=A.max)
            nc.vector.tensor_reduce(
                gs3, p3[:].rearrange("p (g c) -> p g c", g=JG),
                axis=AX.X, op=A.max)
            nc.vector.tensor_reduce(
                gs4, p4[:].rearrange("p (g c) -> p g c", g=JG),
                axis=AX.X, op=A.max)
            nc.vector.tensor_reduce(
                gs, gpad[:, 0:320].rearrange("p (c g) -> p g c", c=5),
                axis=AX.X, op=A.max)

            gm8 = med.tile([P, TB * 8], f32, tag="gm8")
            for j in range(TB):
                nc.vector.max(out=gm8[:, j * 8:(j + 1) * 8],
                              in_=gs[:, j * G:(j + 1) * G])
            dpad = med.tile([P, 1024], f32, tag="dpad")
            d = dpad[:, 0:JG]
            goff = dpad[:, 512:512 + JG]
            nc.vector.tensor_tensor(
                d.rearrange("p (j g) -> p j g", j=TB),
                gs.rearrange("p (j g) -> p j g", j=TB),
                gm8[:, 3::8].rearrange("p (j o) -> p j o", o=1)
                .to_broadcast([P, TB, G]),
                op=A.is_lt)
            nc.vector.tensor_scalar(goff, d, -BIG, None, op0=A.mult)

            nc.gpsimd.tensor_tensor(
                swbm[:].rearrange("p (j g e) -> p j g e", j=TB, g=G),
                swb[:].rearrange("p (j g e) -> p j g e", j=TB, g=G),
                goff.rearrange("p (j g) -> p j g", j=TB)
                .to_broadcast([P, TB, G, EG]),
                op=A.add)
            state[i] = (scores, swbm)

        def ph2a(i):
            """v8b (nudge issued separately at end of iteration)"""
            scores, swbm = state[i]
            v8b = sel.tile([P, TB * 8], f32, tag="v8b")
            for j in range(TB):
                nc.vector.max(out=v8b[:, j * 8:(j + 1) * 8],
                              in_=swbm[:, j * E:(j + 1) * E])
            state[i] = (scores, swbm, v8b)

        def ph2n(i):
            """nudged threshold from v8b (late: v8b has drained)"""
            scores, swbm, v8b = state[i]
            c = 1.5 * 2.0 ** -23
            pad = sel.tile([P, 768], f32, tag="nudgepad")
            t8lo = pad[:, 0:TB]
            t8hi = pad[:, 256:256 + TB]
            nt8p = pad[:, 512:512 + TB]
            nc.vector.tensor_scalar(t8lo, v8b[:, 7::8], c - 1.0, None,
                                    op0=A.mult)
            nc.vector.tensor_scalar(t8hi, v8b[:, 7::8], -c - 1.0, None,
                                    op0=A.mult)
            nc.vector.tensor_tensor(nt8p, t8lo, t8hi, op=A.max)
            state[i] = (scores, swbm, nt8p)

        def ph2b(i):
            """sign select + s = scores * sgn"""
            scores, swbm, nt8p = state[i]
            for j in range(TB):
                nc.scalar.activation(
                    swbm[:, j * E:(j + 1) * E], swbm[:, j * E:(j + 1) * E],
                    AF.Sign, bias=nt8p[:, j:j + 1], scale=1.0)
            s = pool_s.tile([P, W], f32, tag="s")
            nc.gpsimd.tensor_tensor(s[:], scores[:], swbm[:], op=A.mult)
            state[i] = s

        def ph3(i):
            """final top-8 + indices"""
            s = state.pop(i)
            v8u = out.tile([P, TB * 8], f32, tag="v8u")
            idx8 = out.tile([P, TB * 8], mybir.dt.int32, tag="idx8")
            for j in range(TB):
                nc.vector.max(out=v8u[:, j * 8:(j + 1) * 8],
                              in_=s[:, j * E:(j + 1) * E])
            for j in range(TB):
                nc.vector.max_index(
                    out=idx8[:, j * 8:(j + 1) * 8].bitcast(u32),
                    in_max=v8u[:, j * 8:(j + 1) * 8],
                    in_values=s[:, j * E:(j + 1) * E])
            ssum = out.tile([P, TB], f32, tag="ssum")
            nc.vector.tensor_reduce(
                ssum[:], v8u[:].rearrange("p (j k) -> p j k", j=TB),
                axis=AX.X, op=A.add)
            state[("o", i)] = (v8u, idx8, ssum)

        def ph3n(i):
            """reciprocal chain (late: ssum has drained)"""
            v8u, idx8, ssum = state.pop(("o", i))
            ssum4 = out.tile([P, TB], f32, tag="ssum4")
            nc.vector.tensor_scalar(ssum4[:], ssum[:], 0.4, None, op0=A.mult)
            rec = out.tile([P, TB], f32, tag="rec")
            nc.vector.reciprocal(rec[:], ssum4[:])
            state[("o", i)] = (v8u, idx8, rec)

        def ph3b(i):
            v8u, idx8, rec = state.pop(("o", i))
            rows = slice(i * P * TB, (i + 1) * P * TB)
            vals8 = out.tile([P, TB * 8], f32, tag="vals8")
            nc.gpsimd.tensor_tensor(
                vals8[:].rearrange("p (j k) -> p j k", j=TB),
                v8u[:].rearrange("p (j k) -> p j k", j=TB),
                rec[:].rearrange("p (j o) -> p j o", o=1).to_broadcast([P, TB, 8]),
                op=A.mult)
            oi = idx_d[rows, :].rearrange("(j p) k -> p j k", p=P)
            ov = vals_d[rows, :].rearrange("(j p) k -> p j k", p=P)
            nc.sync.dma_start(oi, idx8[:].rearrange("p (j k) -> p j k", j=TB))
            nc.sync.dma_start(ov, vals8[:].rearrange("p (j k) -> p j k", j=TB))

        def live(j):
            return 0 <= j < n_groups

        for k in range(n_groups + 4):
            if live(k):
                ph1f(k)
            if live(k - 2):
                ph2b(k - 2)
            if live(k - 3):
                ph3(k - 3)
            if live(k - 4):
                ph3b(k - 4)
            if live(k - 1):
                ph2a(k - 1)
            if live(k):
                ph1(k)
            if live(k - 1):
                ph2n(k - 1)
            if live(k - 3):
                ph3n(k - 3)

    nc.compile()
    return nc


_NC_CACHE = {}


def _get_nc(n_tokens: int):
    if n_tokens not in _NC_CACHE:
        _NC_CACHE[n_tokens] = build_bass(n_tokens)
    return _NC_CACHE[n_tokens]


def _host_tiles(bias):
    biasb = np.ascontiguousarray(
        np.broadcast_to(np.tile(bias, TB)[None, :], (P, TB * E)).astype(np.float32))
    return biasb


def run_spmd(nc, logits, biasb, trace=False):
    from concourse import bass_utils

    n = logits.shape[0] // N_CORES
    in_maps = [
        {"logits": np.ascontiguousarray(logits[c * n:(c + 1) * n]),
         "biasb": biasb}
        for c in range(N_CORES)
    ]
    res = bass_utils.run_bass_kernel_spmd(nc, in_maps, list(range(N_CORES)),
                                          trace=trace)
    idx = np.concatenate([r["idx"] for r in res.results], axis=0)
    vals = np.concatenate([r["vals"] for r in res.results], axis=0)
    return (idx.astype(np.int32), vals.astype(np.float32)), res


def kernel(logits, e_score_correction_bias):
    logits = np.asarray(logits, dtype=np.float32)
    bias = np.asarray(e_score_correction_bias, dtype=np.float32)
    assert logits.shape == (T_FULL, E)
    biasb = _host_tiles(bias)
    nc = _get_nc(T_CORE)
    (idx, vals), _ = run_spmd(nc, logits, biasb)
    return idx, vals
